# revision 2
# baseline (speedup 1.0000x reference)
"""Trainium2 Bass kernel for the 2-layer TransformerConv GNN + MLP head.

Contract: kernel(**inputs) takes FULL inputs, returns FULL [N, 2] output,
running on 8 NeuronCores via run_bass_kernel_spmd.

Design:
- Edges sharded by dst tile (disjoint segment sums, no all-reduce).
- Per-edge Q is expanded on the TensorEngine (ohT @ q_tile, ohT = PE
  transpose of the dst one-hot) -- Q is never gathered from DRAM.
- KV tables are bf16 and SHARED across cores: each core projects only its
  own nodes, a shared-output AllGather materializes ONE chip-wide table,
  halving HBM write traffic and removing all replicated projection work.
- dma_gather calls capped at 1024 indices (hard device limit), batched
  across TB tiles, spread over 4 SWDGE queues with an enlarged descriptor
  ring so transfers from different queues overlap.
- Edge math is bf16 with few, large engine ops (G-chunk batching); the
  k+e / v+e adds are fused into one broadcast add.
"""

import sys

sys.path.insert(0, "/opt/trn_rl_repo")

import os

import numpy as np
import ml_dtypes

import concourse.bacc as bacc
import concourse.bass as bass
import concourse.mybir as mybir
import concourse.tile as tile
from concourse.bass_utils import run_bass_kernel_spmd
from concourse.masks import make_identity

P = 128
NCORES = 8
FP = mybir.dt.float32
BF = mybir.dt.bfloat16
BF_NP = ml_dtypes.bfloat16

HEADS = 4
TB = 2            # tiles per dma_gather batch
G = int(os.environ.get('KB2_G', '4'))  # chunks per DVE slab group
TG = 8            # tiles per epilogue batch
MAXC = 8          # dma_gather hard limit: 1024 indices per call
PSB = int(os.environ.get('KB2_PSB', '2'))   # psum bufs for group tiles
KVB = int(os.environ.get('KB2_KVB', '2'))   # kv gather tile bufs


# ----------------------------------------------------------------------------
# host-side preprocessing
# ----------------------------------------------------------------------------

def _wrap_idx(a):
    """[S] int16 -> dma_gather wrapped layout [128, S//16]: index i lands at
    [i % 16, i // 16], replicated x8 down the partitions."""
    S = a.shape[0]
    w = np.ascontiguousarray(a.reshape(S // 16, 16).T)
    return np.tile(w, (8, 1))


def host_prep(x, edge_index, edge_attr, n_nodes, n_edges, fe):
    t_total = -(-n_nodes // P)
    t_core = -(-t_total // NCORES)
    t_all = t_core * NCORES
    n_pad = t_all * P
    n_core = t_core * P
    half = (n_pad // 2 + P - 1) // P * P
    assert half < 32768 and n_pad - half < 32768

    src = np.asarray(edge_index[0], dtype=np.int64)
    dst = np.asarray(edge_index[1], dtype=np.int64)
    ea = np.asarray(edge_attr, dtype=np.float32)

    tile_of = dst // P
    key = (tile_of * 2 + (src >= half)).astype(np.int64)
    order = np.argsort(key, kind="stable")
    counts = np.bincount(key, minlength=t_all * 2)
    cl = int(-(-counts[0::2].max() // P))
    ch = int(-(-counts[1::2].max() // P))
    ct = cl + ch
    cap = ct * P

    sorted_keys = key[order]
    grp_starts = np.concatenate(([0], np.cumsum(counts)[:-1]))
    pos = np.arange(n_edges) - grp_starts[sorted_keys]
    dest = (sorted_keys // 2) * cap + (sorted_keys % 2) * (cl * P) + pos

    slot_edge = np.full(t_all * cap, -1, np.int64)
    slot_edge[dest] = order
    valid = slot_edge >= 0
    e_idx = np.where(valid, slot_edge, 0)
    src_s = src[e_idx]
    dst_s = dst[e_idx]

    kvidx = np.where(valid, np.where(src_s < half, src_s, src_s - half), 0)
    kvidx = kvidx.astype(np.int16).reshape(t_all, cap)
    t_arr = np.repeat(np.arange(t_all), cap)
    dstrel = np.where(valid, dst_s - t_arr * P, -1).astype(BF_NP)
    dstrel = dstrel.reshape(t_all, ct, P)           # [T, chunk, edge-in-chunk]
    dstrel_t = np.ascontiguousarray(dstrel.transpose(0, 2, 1))  # [T, P, ct]
    ea_slots = np.where(valid[:, None], ea[e_idx], 0).astype(np.float32)
    eaT = np.ascontiguousarray(
        ea_slots.reshape(t_all, cap, fe).transpose(0, 2, 1)
    ).astype(BF_NP)                                  # [T, FE, cap]

    x_pad = np.zeros((n_pad, x.shape[1]), np.float32)
    x_pad[:n_nodes] = x
    xT_all = np.ascontiguousarray(x_pad.T).astype(BF_NP)

    percore = []
    for c in range(NCORES):
        ts = slice(c * t_core, (c + 1) * t_core)
        cols = []
        for g0 in range(0, t_core, TB):
            gts = range(c * t_core + g0, c * t_core + min(g0 + TB, t_core))
            lo = np.concatenate([kvidx[t, : cl * P] for t in gts])
            hi = np.concatenate([kvidx[t, cl * P:] for t in gts])
            cols.append(_wrap_idx(lo))
            cols.append(_wrap_idx(hi))
        percore.append(
            dict(
                xT_own=np.ascontiguousarray(
                    xT_all[:, c * n_core:(c + 1) * n_core]),
                eaT=np.ascontiguousarray(eaT[ts]),
                kvidx=np.ascontiguousarray(np.concatenate(cols, axis=1)),
                dstrel=np.ascontiguousarray(
                    dstrel_t[ts].transpose(1, 0, 2).reshape(P, -1)),
                dstrel_row=np.ascontiguousarray(
                    dstrel[ts].reshape(t_core, 1, cap)),
            )
        )
    dcfg = dict(
        t_core=t_core, cl=cl, ch=ch, half=half, n_pad=n_pad, n_core=n_core,
        fn=x.shape[1], fe=fe, h=HEADS,
    )
    return percore, dcfg


def pack_weights(i):
    f32 = lambda a: np.ascontiguousarray(np.asarray(a, np.float32))
    bf = lambda a: np.ascontiguousarray(np.asarray(a, np.float32)).astype(BF_NP)
    cat = lambda *a: np.concatenate([np.asarray(x, np.float32) for x in a],
                                    axis=-1)
    w = dict(
        wkv1=bf(cat(i["Wk1"], i["Wv1"])),
        bkv1=bf(cat(i["bk1"], i["bv1"])[None, :]),
        wqs1=bf(cat(i["Wq1"], i["Ws1"])),
        bqs1=bf(cat(i["bq1"], i["bs1"])[None, :]),
        we1=bf(i["We1"]),
        wkv2=bf(cat(i["Wk2"], i["Wv2"])),
        bkv2=bf(cat(i["bk2"], i["bv2"])[None, :]),
        wqs2=bf(cat(i["Wq2"], i["Ws2"])),
        bqs2=bf(cat(i["bq2"], i["bs2"])[None, :]),
        we2=bf(i["We2"]),
        w3=bf(i["W3"]), b3=f32(i["b3"])[:, None],
        w4=bf(i["W4"]), b4=f32(i["b4"])[:, None],
    )
    flags = dict(
        bkv1_nz=bool(np.any(np.asarray(i["bk1"])) or np.any(np.asarray(i["bv1"]))),
        bqs1_nz=bool(np.any(np.asarray(i["bq1"])) or np.any(np.asarray(i["bs1"]))),
        bkv2_nz=bool(np.any(np.asarray(i["bk2"])) or np.any(np.asarray(i["bv2"]))),
        bqs2_nz=bool(np.any(np.asarray(i["bq2"])) or np.any(np.asarray(i["bs2"]))),
    )
    return w, flags


# ----------------------------------------------------------------------------
# device program
# ----------------------------------------------------------------------------

def _edge_layer(nc, tc, pool, psum, cfg, consts, lay):
    """One TransformerConv edge pass over this core's tiles."""
    t_core, cl, ch = cfg["t_core"], cfg["cl"], cfg["ch"]
    ct = cl + ch
    fe, H = cfg["fe"], cfg["h"]
    c = lay["c"]
    hc = H * c
    iota_row = consts["iota_row"]
    ident = consts["ident"]
    iota_col, ones_row = consts["iota_col"], consts["ones_row"]
    dstrel_row_d = consts["dstrel_row_d"]
    kvidx_sb, dstrel_sb = consts["kvidx"], consts["dstrel"]
    scale = 1.0 / float(np.sqrt(c))
    kv_lo, kv_hi = lay["kv_lo"], lay["kv_hi"]
    q_all, skip_all = lay["q_all"], lay["skip_all"]
    We_sb = lay["We_sb"]
    h_res = lay["h_res"]
    G = lay.get("G", 4)
    BCB = lay.get("bcb", 2)
    gsec = ct * 8                            # idx cols per tile

    groups = [(g0, min(G, cl - g0)) for g0 in range(0, cl, G)]
    groups += [(g0, min(G, ct - g0)) for g0 in range(cl, ct, G)]

    agg_grp = None
    qctr = 0
    for tb in range(0, t_core, TB):
        tn = min(TB, t_core - tb)
        kvlo_t = pool.tile([P, TB * cl, 2 * hc], BF, tag="kvlo", bufs=KVB)
        kvhi_t = pool.tile([P, TB * ch, 2 * hc], BF, tag="kvhi", bufs=KVB)
        i0 = tb * gsec

        def emit_gathers(out_tile, table, ibase, nch):
            nonlocal qctr
            for s0 in range(0, nch, MAXC):
                n = min(MAXC, nch - s0)
                nc.gpsimd.dma_gather(
                    out_tile[:, s0:s0 + n, :], table,
                    kvidx_sb[:, ibase + s0 * 8:ibase + (s0 + n) * 8],
                    n * P, n * P, 2 * hc, queue_num=qctr % 4)
                qctr += 1

        emit_gathers(kvlo_t, kv_lo, i0, tn * cl)
        emit_gathers(kvhi_t, kv_hi, i0 + tn * cl * 8, tn * ch)

        for tt in range(tn):
            t = tb + tt
            eaT_t = pool.tile([fe, ct * P], BF, tag="eaT")
            nc.sync.dma_start(out=eaT_t[:], in_=lay["eaT_dram"][t])
            dr_t = pool.tile([1, ct * P], BF, tag="dr_t")
            nc.sync.dma_start(out=dr_t[:], in_=dstrel_row_d[t])

            agg_ps = psum.tile([P, H * (c + 1)], FP, space="PSUM", tag="agg")
            first = True
            for g0, gn in groups:
                # edge-attr projection e = eaT^T @ We  (PE)
                e_ps = psum.tile([P, G * hc], FP, space="PSUM", tag="e_ps",
                                 bufs=PSB)
                for j in range(gn):
                    nc.tensor.matmul(
                        out=e_ps[:, j * hc:(j + 1) * hc],
                        lhsT=eaT_t[:, (g0 + j) * P:(g0 + j + 1) * P],
                        rhs=We_sb[:], start=True, stop=True)
                # dst one-hot oh[e, n], one DVE op per group
                oh = pool.tile([P, G * P], BF, tag="oh")
                nc.vector.tensor_tensor(
                    out=oh[:].rearrange("p (g f) -> p g f", g=G)[:, 0:gn, :],
                    in0=iota_row[:, 0:G * P].rearrange("p (g f) -> p g f",
                                                       g=G)[:, 0:gn, :],
                    in1=dstrel_sb[:, t * ct + g0: t * ct + g0 + gn][:, :, None]
                        .to_broadcast([P, gn, P]),
                    op=mybir.AluOpType.is_equal)
                # ohT[n, e] via dstrel_row broadcast (PE) + is_equal (DVE)
                bc_ps = psum.tile([P, G * P], FP, space="PSUM", tag="bc_ps",
                                  bufs=BCB)
                for b0 in range(0, gn, 4):
                    bn = min(4, gn - b0)
                    nc.tensor.matmul(
                        out=bc_ps[:, b0 * P:(b0 + bn) * P],
                        lhsT=ones_row[0:1, :],
                        rhs=dr_t[0:1, (g0 + b0) * P:(g0 + b0 + bn) * P],
                        start=True, stop=True)
                ohT_sb = pool.tile([P, G * P], BF, tag="ohT_sb")
                nc.vector.tensor_scalar(
                    out=ohT_sb[:, 0:gn * P], in0=bc_ps[:, 0:gn * P],
                    scalar1=iota_col[:, 0:1], scalar2=None,
                    op0=mybir.AluOpType.is_equal)
                qe_ps = psum.tile([P, G * hc], FP, space="PSUM", tag="qe_ps",
                                  bufs=PSB)
                for j in range(gn):
                    nc.tensor.matmul(
                        out=qe_ps[:, j * hc:(j + 1) * hc],
                        lhsT=ohT_sb[:, j * P:(j + 1) * P],
                        rhs=q_all[:, t * hc:(t + 1) * hc],
                        start=True, stop=True)
                # e -> sbuf bf16 (Act), then [k|v] + e in ONE broadcast add
                e_sb2 = pool.tile([P, G * 2 * hc], BF, tag="e_sb2")
                e24 = e_sb2[:].rearrange("p (g k f) -> p g k f", g=G, k=2)
                e_ps3 = e_ps[:].rearrange("p (g f) -> p g f", g=G)[:, 0:gn, :]
                nc.scalar.activation(
                    out=e24[:, 0:gn, 0, :], in_=e_ps3,
                    func=mybir.ActivationFunctionType.Copy)
                nc.scalar.activation(
                    out=e24[:, 0:gn, 1, :], in_=e_ps3,
                    func=mybir.ActivationFunctionType.Copy)
                if g0 < cl:
                    assert g0 + gn <= cl
                    kv_sl = kvlo_t[:, tt * cl + g0: tt * cl + g0 + gn, :]
                else:
                    kv_sl = kvhi_t[:, tt * ch + (g0 - cl): tt * ch + (g0 - cl) + gn, :]
                kve = pool.tile([P, G * 2 * hc], BF, tag="kve")
                kve4 = kve[:].rearrange("p (g k f) -> p g k f", g=G, k=2)
                nc.vector.tensor_tensor(
                    out=kve4[:, 0:gn],
                    in0=kv_sl[:].rearrange("p g (k f) -> p g k f", k=2),
                    in1=e24[:, 0:gn],
                    op=mybir.AluOpType.add)
                # kq = ke * q_e ; logits = reduce over c
                qe_sb = pool.tile([P, G * hc], BF, tag="qe_sb")
                nc.scalar.activation(
                    out=qe_sb[:, 0:gn * hc], in_=qe_ps[:, 0:gn * hc],
                    func=mybir.ActivationFunctionType.Copy)
                kq = pool.tile([P, G * hc], BF, tag="kq")
                nc.vector.tensor_tensor(
                    out=kq[:].rearrange("p (g f) -> p g f", g=G)[:, 0:gn, :],
                    in0=kve4[:, 0:gn, 0, :],
                    in1=qe_sb[:].rearrange("p (g f) -> p g f", g=G)[:, 0:gn, :],
                    op=mybir.AluOpType.mult)
                lg = pool.tile([P, G * H], FP, tag="lg")
                nc.vector.reduce_sum(
                    out=lg[:].rearrange("p (g h) -> p g h", g=G)[:, 0:gn, :],
                    in_=kq[:].rearrange("p (g h w) -> p g h w", g=G, h=H)[:, 0:gn],
                    axis=mybir.AxisListType.X)
                p_t = pool.tile([P, G * H], BF, tag="p_t")
                nc.scalar.activation(
                    out=p_t[:, 0:gn * H], in_=lg[:, 0:gn * H],
                    func=mybir.ActivationFunctionType.Exp, scale=scale)
                # pva = [p*v | p]
                pva = pool.tile([P, G * H * (c + 1)], BF, tag="pva")
                pva4 = pva[:].rearrange("p (g h w) -> p g h w", g=G, h=H)
                p3 = p_t[:].rearrange("p (g h) -> p g h", g=G)
                nc.vector.tensor_tensor(
                    out=pva4[:, 0:gn, :, 0:c],
                    in0=kve4[:, 0:gn, 1, :].rearrange("p g (h w) -> p g h w",
                                                      h=H),
                    in1=p3[:, 0:gn, :, None].to_broadcast([P, gn, H, c]),
                    op=mybir.AluOpType.mult)
                nc.scalar.activation(out=pva4[:, 0:gn, :, c], in_=p3[:, 0:gn, :],
                                     func=mybir.ActivationFunctionType.Copy)
                for j in range(gn):
                    nc.tensor.matmul(
                        out=agg_ps[:],
                        lhsT=oh[:, j * P:(j + 1) * P],
                        rhs=pva[:, j * H * (c + 1):(j + 1) * H * (c + 1)],
                        start=first, stop=(g0 + j == ct - 1))
                    first = False

            # epilogue, batched per TG tiles
            tg = t % TG
            if tg == 0:
                agg_grp = pool.tile([P, TG * H * (c + 1)], FP, tag="agg_grp")
            nc.scalar.activation(
                out=agg_grp[:, tg * H * (c + 1):(tg + 1) * H * (c + 1)],
                in_=agg_ps[:], func=mybir.ActivationFunctionType.Copy)
            if tg == TG - 1 or t == t_core - 1:
                n = tg + 1
                t0 = t - tg
                a4 = agg_grp[:].rearrange("p (t h w) -> p t h w", t=TG, h=H)
                sp = pool.tile([P, TG * H], FP, tag="sp")
                nc.vector.tensor_scalar(
                    out=sp[:, 0:n * H],
                    in0=a4[:, 0:n, :, c].rearrange("p t h -> p (t h)"),
                    scalar1=1e-30, scalar2=float(H),
                    op0=mybir.AluOpType.add, op1=mybir.AluOpType.mult)
                rs = pool.tile([P, TG * H], FP, tag="rs")
                nc.vector.reciprocal(out=rs[:, 0:n * H], in_=sp[:, 0:n * H])
                nc.vector.tensor_tensor(
                    out=a4[:, 0:n, :, 0:c], in0=a4[:, 0:n, :, 0:c],
                    in1=rs[:].rearrange("p (t h) -> p t h", t=TG)[:, 0:n, :, None]
                        .to_broadcast([P, n, H, c]),
                    op=mybir.AluOpType.mult)
                hsum = pool.tile([P, TG * c], FP, tag="hsum")
                nc.vector.reduce_sum(
                    out=hsum[:].rearrange("p (t w) -> p t w", t=TG)[:, 0:n],
                    in_=agg_grp[:].rearrange("p (t h w) -> p t w h", t=TG,
                                             h=H)[:, 0:n, 0:c, :],
                    axis=mybir.AxisListType.X)
                nc.vector.tensor_tensor(
                    out=hsum[:, 0:n * c], in0=hsum[:, 0:n * c],
                    in1=skip_all[:, t0 * c:(t0 + n) * c],
                    op=mybir.AluOpType.add)
                nc.scalar.activation(
                    out=h_res[:, t0 * c:(t0 + n) * c], in_=hsum[:, 0:n * c],
                    func=mybir.ActivationFunctionType.Relu)


def build_device(dcfg):
    phases = os.environ.get("KB2_PHASES", "ACDFG")
    t_core, cl, ch = dcfg["t_core"], dcfg["cl"], dcfg["ch"]
    ct = cl + ch
    n_pad, n_core = dcfg["n_pad"], dcfg["n_core"]
    fn, fe, H = dcfg["fn"], dcfg["fe"], dcfg["h"]
    c1, c2 = 32, 16
    ncls = 2
    hc1, hc2 = H * c1, H * c2
    hid = 2 * c2
    half = dcfg["half"]

    nc = bacc.Bacc("TRN2", target_bir_lowering=False, debug=False,
                   num_devices=NCORES, num_swdge_queues=4,
                   dynamic_dma_scratch_size=32768)

    def param(name, shape, dtype=FP, out=False):
        return nc.declare_dram_parameter(name, list(shape), dtype, isOutput=out)

    xT_own_d = param("xT_own", [fn, n_core], BF)
    eaT_d = param("eaT", [t_core, fe, ct * P], BF)
    kvidx_d = param("kvidx", [P, t_core * ct * 8], mybir.dt.int16)
    dstrel_d = param("dstrel", [P, t_core * ct], BF)
    dstrel_row_d = param("dstrel_row", [t_core, 1, ct * P], BF)
    wkv1_d = param("wkv1", [fn, 2 * hc1], BF)
    bkv1_d = param("bkv1", [1, 2 * hc1], BF)
    wqs1_d = param("wqs1", [fn, hc1 + c1], BF)
    bqs1_d = param("bqs1", [1, hc1 + c1], BF)
    we1_d = param("we1", [fe, hc1], BF)
    wkv2_d = param("wkv2", [c1, 2 * hc2], BF)
    bkv2_d = param("bkv2", [1, 2 * hc2], BF)
    wqs2_d = param("wqs2", [c1, hc2 + c2], BF)
    bqs2_d = param("bqs2", [1, hc2 + c2], BF)
    we2_d = param("we2", [fe, hc2], BF)
    w3_d = param("w3", [c2, hid], BF)
    b3_d = param("b3", [hid, 1])
    w4_d = param("w4", [hid, ncls], BF)
    b4_d = param("b4", [ncls, 1])
    out_d = param("out", [ncls, n_core], out=True)

    bias_flags = dcfg.get("bias_flags", dict(
        bkv1_nz=False, bqs1_nz=False, bkv2_nz=False, bqs2_nz=False))

    with tile.TileContext(nc) as tc:
        with (
            tc.tile_pool(name="res", bufs=1) as res,
            tc.tile_pool(name="sbuf", bufs=2) as pool,
            tc.tile_pool(name="dram", bufs=1, space="DRAM") as dram,
        ):
            # ---- constants
            ident_f = res.tile([P, P], FP)
            make_identity(nc, ident_f[:])
            ident = res.tile([P, P], BF)
            nc.vector.tensor_copy(out=ident[:], in_=ident_f[:])
            ones_row = res.tile([1, P], BF)
            nc.vector.memset(ones_row[:], 1.0)
            iota_row = res.tile([P, 6 * P], BF)
            nc.gpsimd.iota(iota_row[:, 0:P], pattern=[[1, P]], base=0,
                           channel_multiplier=0,
                           allow_small_or_imprecise_dtypes=True)
            for g in range(1, 6):
                nc.vector.tensor_copy(out=iota_row[:, g * P:(g + 1) * P],
                                      in_=iota_row[:, 0:P])
            iota_col = res.tile([P, 1], FP)
            nc.gpsimd.iota(iota_col[:], pattern=[[0, 1]], base=0,
                           channel_multiplier=1,
                           allow_small_or_imprecise_dtypes=True)
            kvidx_sb = res.tile([P, t_core * ct * 8], mybir.dt.int16)
            nc.sync.dma_start(out=kvidx_sb[:], in_=kvidx_d[:])
            dstrel_sb = res.tile([P, t_core * ct], BF)
            nc.sync.dma_start(out=dstrel_sb[:], in_=dstrel_d[:])

            def load_w(d, shape, tag, dt=BF):
                t = res.tile(list(shape), dt, tag=tag)
                nc.sync.dma_start(out=t[:], in_=d[:])
                return t

            wkv1 = load_w(wkv1_d, [fn, 2 * hc1], "wkv1")
            bkv1 = load_w(bkv1_d, [1, 2 * hc1], "bkv1")
            wqs1 = load_w(wqs1_d, [fn, hc1 + c1], "wqs1")
            bqs1 = load_w(bqs1_d, [1, hc1 + c1], "bqs1")
            we1 = load_w(we1_d, [fe, hc1], "we1")
            wkv2 = load_w(wkv2_d, [c1, 2 * hc2], "wkv2")
            bkv2 = load_w(bkv2_d, [1, 2 * hc2], "bkv2")
            wqs2 = load_w(wqs2_d, [c1, hc2 + c2], "wqs2")
            bqs2 = load_w(bqs2_d, [1, hc2 + c2], "bqs2")
            we2 = load_w(we2_d, [fe, hc2], "we2")
            w3 = load_w(w3_d, [c2, hid], "w3")
            b3 = load_w(b3_d, [hid, 1], "b3", FP)
            w4 = load_w(w4_d, [hid, ncls], "w4")
            b4 = load_w(b4_d, [ncls, 1], "b4", FP)

            q1_all = res.tile([P, t_core * hc1], BF)
            skip1_all = res.tile([P, t_core * c1], FP)
            q2_all = res.tile([P, t_core * hc2], BF)
            skip2_all = res.tile([P, t_core * c2], FP)
            h1_res = res.tile([P, t_core * c1], BF)
            h2_res = res.tile([P, t_core * c2], FP)
            h2T_res = res.tile([c2, t_core * P], BF)

            # ---- DRAM: per-core shards + chip-shared gathered tables
            kv1_shard = dram.tile([n_core, 2 * hc1], BF)
            kv2_shard = dram.tile([n_core, 2 * hc2], BF)
            kv1_all = dram.tile([n_pad, 2 * hc1], BF, addr_space="Shared")
            kv2_all = dram.tile([n_pad, 2 * hc2], BF, addr_space="Shared")

            consts = dict(iota_row=iota_row, ident=ident, iota_col=iota_col,
                          ones_row=ones_row, kvidx=kvidx_sb, dstrel=dstrel_sb,
                          dstrel_row_d=dstrel_row_d)

            # ---- phase A: layer-1 projections for own nodes
            AB = 8
            if "A" not in phases:
                nc.vector.memset(q1_all[:], 0.0)
                nc.vector.memset(skip1_all[:], 0.0)
            if "A" in phases:
             with tc.tile_pool(name="psumA", bufs=2, space="PSUM") as psum:
                for t0 in range(0, t_core, AB):
                    n = min(AB, t_core - t0)
                    xg = pool.tile([fn, AB * P], BF, tag="xg")
                    nc.sync.dma_start(
                        out=xg[:, 0:n * P],
                        in_=xT_own_d[:, t0 * P:(t0 + n) * P])
                    stg = pool.tile([P, AB * 2 * hc1], BF, tag="stgA")
                    for j in range(n):
                        t = t0 + j
                        ps = psum.tile([P, 2 * hc1 + hc1 + c1], FP,
                                       space="PSUM", tag="psA")
                        nc.tensor.matmul(
                            out=ps[:, 0:2 * hc1],
                            lhsT=xg[:, j * P:(j + 1) * P], rhs=wkv1[:],
                            start=True, stop=not bias_flags["bkv1_nz"])
                        if bias_flags["bkv1_nz"]:
                            nc.tensor.matmul(
                                out=ps[:, 0:2 * hc1], lhsT=ones_row[0:1, :],
                                rhs=bkv1[0:1, :], start=False, stop=True)
                        nc.tensor.matmul(
                            out=ps[:, 2 * hc1:],
                            lhsT=xg[:, j * P:(j + 1) * P], rhs=wqs1[:],
                            start=True, stop=not bias_flags["bqs1_nz"])
                        if bias_flags["bqs1_nz"]:
                            nc.tensor.matmul(
                                out=ps[:, 2 * hc1:], lhsT=ones_row[0:1, :],
                                rhs=bqs1[0:1, :], start=False, stop=True)
                        if j % 2 == 0:
                            nc.vector.tensor_copy(
                                out=stg[:, j * 2 * hc1:(j + 1) * 2 * hc1],
                                in_=ps[:, 0:2 * hc1])
                            nc.scalar.activation(
                                out=q1_all[:, t * hc1:(t + 1) * hc1],
                                in_=ps[:, 2 * hc1:3 * hc1],
                                func=mybir.ActivationFunctionType.Copy)
                        else:
                            nc.scalar.activation(
                                out=stg[:, j * 2 * hc1:(j + 1) * 2 * hc1],
                                in_=ps[:, 0:2 * hc1],
                                func=mybir.ActivationFunctionType.Copy)
                            nc.vector.tensor_copy(
                                out=q1_all[:, t * hc1:(t + 1) * hc1],
                                in_=ps[:, 2 * hc1:3 * hc1])
                        nc.vector.tensor_copy(
                            out=skip1_all[:, t * c1:(t + 1) * c1],
                            in_=ps[:, 3 * hc1:3 * hc1 + c1])
                    nc.sync.dma_start(
                        out=kv1_shard[t0 * P:(t0 + n) * P, :].rearrange(
                            "(t p) w -> p t w", p=P),
                        in_=stg[:, 0:n * 2 * hc1].rearrange(
                            "p (t w) -> p t w", t=n))

             nc.gpsimd.collective_compute(
                 "AllGather", mybir.AluOpType.bypass,
                 replica_groups=[list(range(NCORES))],
                 ins=[kv1_shard[:].opt()], outs=[kv1_all[:].opt()])

            # ---- phase C: layer-1 edge pass
            if "C" not in phases:
                nc.vector.memset(h1_res[:], 0.0)
            if "C" in phases:
             with tc.tile_pool(name="psumC", bufs=2, space="PSUM") as psum:
                _edge_layer(nc, tc, pool, psum, dcfg, consts, dict(
                    c=c1, q_all=q1_all, skip_all=skip1_all,
                    kv_lo=kv1_all[0:half, :], kv_hi=kv1_all[half:, :],
                    We_sb=we1, eaT_dram=eaT_d, h_res=h1_res[:],
                    G=int(os.environ.get("KB2_G1", "4")), bcb=2))

            # ---- phase D: h1 transposes, q2/skip2 + kv2 projections
            DB = 8
            if "D" not in phases:
                nc.vector.memset(q2_all[:], 0.0)
                nc.vector.memset(skip2_all[:], 0.0)
            if "D" in phases:
             with tc.tile_pool(name="psumD", bufs=2, space="PSUM") as psum:
                for t0 in range(0, t_core, DB):
                    n = min(DB, t_core - t0)
                    hg = pool.tile([c1, DB * P], BF, tag="hT_g")
                    for j in range(n):
                        t = t0 + j
                        hT_ps = psum.tile([c1, P], BF, space="PSUM",
                                          tag="hT_ps")
                        nc.tensor.transpose(
                            out=hT_ps[:], in_=h1_res[:, t * c1:(t + 1) * c1],
                            identity=ident[:])
                        if j % 2 == 0:
                            nc.vector.tensor_copy(
                                out=hg[:, j * P:(j + 1) * P], in_=hT_ps[:])
                        else:
                            nc.scalar.activation(
                                out=hg[:, j * P:(j + 1) * P], in_=hT_ps[:],
                                func=mybir.ActivationFunctionType.Copy)
                    stg = pool.tile([P, DB * 2 * hc2], BF, tag="stgD")
                    for j in range(n):
                        t = t0 + j
                        ps = psum.tile([P, 2 * hc2 + hc2 + c2], FP,
                                       space="PSUM", tag="psD")
                        nc.tensor.matmul(
                            out=ps[:, 0:2 * hc2],
                            lhsT=hg[:, j * P:(j + 1) * P], rhs=wkv2[:],
                            start=True, stop=not bias_flags["bkv2_nz"])
                        if bias_flags["bkv2_nz"]:
                            nc.tensor.matmul(
                                out=ps[:, 0:2 * hc2], lhsT=ones_row[0:1, :],
                                rhs=bkv2[0:1, :], start=False, stop=True)
                        nc.tensor.matmul(
                            out=ps[:, 2 * hc2:],
                            lhsT=hg[:, j * P:(j + 1) * P], rhs=wqs2[:],
                            start=True, stop=not bias_flags["bqs2_nz"])
                        if bias_flags["bqs2_nz"]:
                            nc.tensor.matmul(
                                out=ps[:, 2 * hc2:], lhsT=ones_row[0:1, :],
                                rhs=bqs2[0:1, :], start=False, stop=True)
                        if j % 2 == 0:
                            nc.vector.tensor_copy(
                                out=stg[:, j * 2 * hc2:(j + 1) * 2 * hc2],
                                in_=ps[:, 0:2 * hc2])
                            nc.scalar.activation(
                                out=q2_all[:, t * hc2:(t + 1) * hc2],
                                in_=ps[:, 2 * hc2:3 * hc2],
                                func=mybir.ActivationFunctionType.Copy)
                        else:
                            nc.scalar.activation(
                                out=stg[:, j * 2 * hc2:(j + 1) * 2 * hc2],
                                in_=ps[:, 0:2 * hc2],
                                func=mybir.ActivationFunctionType.Copy)
                            nc.vector.tensor_copy(
                                out=q2_all[:, t * hc2:(t + 1) * hc2],
                                in_=ps[:, 2 * hc2:3 * hc2])
                        nc.vector.tensor_copy(
                            out=skip2_all[:, t * c2:(t + 1) * c2],
                            in_=ps[:, 3 * hc2:3 * hc2 + c2])
                    nc.sync.dma_start(
                        out=kv2_shard[t0 * P:(t0 + n) * P, :].rearrange(
                            "(t p) w -> p t w", p=P),
                        in_=stg[:, 0:n * 2 * hc2].rearrange(
                            "p (t w) -> p t w", t=n))

             nc.gpsimd.collective_compute(
                 "AllGather", mybir.AluOpType.bypass,
                 replica_groups=[list(range(NCORES))],
                 ins=[kv2_shard[:].opt()], outs=[kv2_all[:].opt()])

            # ---- phase F: layer-2 edge pass
            if "F" not in phases:
                nc.vector.memset(h2_res[:], 0.0)
            if "F" in phases:
             with tc.tile_pool(name="psumF", bufs=2, space="PSUM") as psum:
                _edge_layer(nc, tc, pool, psum, dcfg, consts, dict(
                    c=c2, q_all=q2_all, skip_all=skip2_all,
                    kv_lo=kv2_all[0:half, :], kv_hi=kv2_all[half:, :],
                    We_sb=we2, eaT_dram=eaT_d, h_res=h2_res[:],
                    G=int(os.environ.get("KB2_G2", "6")),
                    bcb=int(os.environ.get("KB2_BCB2", "1"))))
            with tc.tile_pool(name="psumF2", bufs=2, space="PSUM") as psum:
                for t in range(t_core):
                    h2T_ps = psum.tile([c2, P], FP, space="PSUM", tag="h2T_ps")
                    nc.tensor.transpose(
                        out=h2T_ps[:], in_=h2_res[:, t * c2:(t + 1) * c2],
                        identity=ident_f[:])
                    nc.vector.tensor_copy(
                        out=h2T_res[:, t * P:(t + 1) * P], in_=h2T_ps[:])

            # ---- phase G: dense head (outputs transposed [ncls, n_core])
            CHUNK = 512
            with tc.tile_pool(name="psumG", bufs=2, space="PSUM") as psum:
                for k0 in range(0, n_core, CHUNK):
                    kn = min(CHUNK, n_core - k0)
                    h3_ps = psum.tile([hid, CHUNK], FP, space="PSUM",
                                      tag="h3_ps")
                    nc.tensor.matmul(out=h3_ps[:, 0:kn], lhsT=w3[:],
                                     rhs=h2T_res[:, k0:k0 + kn], start=True,
                                     stop=True)
                    h3_sb = pool.tile([hid, CHUNK], BF, tag="h3_sb")
                    nc.scalar.activation(
                        out=h3_sb[:, 0:kn], in_=h3_ps[:, 0:kn],
                        func=mybir.ActivationFunctionType.Relu,
                        bias=b3[:, 0:1])
                    o_ps = psum.tile([ncls, CHUNK], FP, space="PSUM",
                                     tag="o_ps")
                    nc.tensor.matmul(out=o_ps[:, 0:kn], lhsT=w4[:],
                                     rhs=h3_sb[:, 0:kn], start=True, stop=True)
                    o_sb = pool.tile([ncls, CHUNK], FP, tag="o_sb")
                    nc.vector.tensor_scalar(
                        out=o_sb[:, 0:kn], in0=o_ps[:, 0:kn],
                        scalar1=b4[:, 0:1], scalar2=None,
                        op0=mybir.AluOpType.add)
                    nc.sync.dma_start(out=out_d[:, k0:k0 + kn],
                                      in_=o_sb[:, 0:kn])

    nc.compile()
    return nc


# ----------------------------------------------------------------------------
# entry point
# ----------------------------------------------------------------------------

_CACHE = {}


def _get_nc(dcfg):
    key = tuple(sorted((k, v) for k, v in dcfg.items() if k != "bias_flags"))
    key += tuple(sorted(dcfg.get("bias_flags", {}).items()))
    if key not in _CACHE:
        _CACHE[key] = build_device(dcfg)
    return _CACHE[key]


def kernel(x, edge_index, edge_attr,
           Wq1, bq1, Wk1, bk1, Wv1, bv1, We1, Ws1, bs1,
           Wq2, bq2, Wk2, bk2, Wv2, bv2, We2, Ws2, bs2,
           W3, b3, W4, b4):
    x = np.asarray(x, np.float32)
    n_nodes = x.shape[0]
    n_edges = np.asarray(edge_index).shape[1]
    percore, dcfg = host_prep(x, np.asarray(edge_index),
                              np.asarray(edge_attr, np.float32),
                              n_nodes, n_edges, np.asarray(edge_attr).shape[1])
    weights, flags = pack_weights(dict(
        Wq1=Wq1, bq1=bq1, Wk1=Wk1, bk1=bk1, Wv1=Wv1, bv1=bv1, We1=We1,
        Ws1=Ws1, bs1=bs1, Wq2=Wq2, bq2=bq2, Wk2=Wk2, bk2=bk2, Wv2=Wv2,
        bv2=bv2, We2=We2, Ws2=Ws2, bs2=bs2, W3=W3, b3=b3, W4=W4, b4=b4))
    dcfg["bias_flags"] = flags
    in_maps = [dict(pc, **weights) for pc in percore]
    nc = _get_nc(dcfg)
    res = run_bass_kernel_spmd(nc, in_maps, core_ids=list(range(NCORES)))
    out = np.concatenate([res.results[i]["out"].T for i in range(NCORES)])
    return np.ascontiguousarray(out[:n_nodes])
